# revision 19
# baseline (speedup 1.0000x reference)
"""Trainium2 Bass kernel for nn_AugmentedTensor (per-head bilinear form).

out[b,a,o] = sum_{i,j} h0[b,a,i] * h1[b,a,j] * T[a,i,j,o],  h = concat(x, 1)

Decomposition (i<128 from x0, i=128 is the ones row; same for j):
  main = sum_{i<128,j<128} x0[b,i] x1[b,j] T[a,i,j,o]
       -> stage1 (PE):  r[b,(j,o)] = x0_tile @ Tc[a]          (K=128, N=16384)
       -> stage2a (DVE+ACT): z_j[b,o] = x1[b,j]*r[b,j,o] -> bf16 zbuf
       -> stage2b (PE): identity-stationary matmuls accumulate the 128
          z slices into one PSUM bank as 4 interleaved quarters
          (psum[:, q*128+o] += z_{4r+q}), 4 slices per N=512 matmul.
          This moves the whole j-reduction tree off VectorE onto TensorE
          PSUM accumulation and keeps the PE dense (HAM stays warm).
  uv   = x0 @ T[a,:128,128,:] + x1 @ T[a,128,:128,:] + T[a,128,128,:]
         accumulated directly into quarter 0 of the same PSUM bank.
  final: out = q0+q1+q2+q3 (VectorE, 3 adds) -> DMA.

Sharding: 8 cores; core c -> head a=c>>1, batch half c&1 (2048 rows).
T traffic per core = one head (~8.5 MB) instead of 34 MB replicated.
"""

import numpy as np

BS, A, D, OUT = 4096, 4, 128, 128
NCORES = 8
BH = BS // 2      # batch rows per core
P = 128
NT = BH // P      # 16 tiles per core
NJ = 128          # j slices per tile
NCH = 32          # 512-wide stage-1 chunks per tile

# Hybrid j-split: j in [0, NJB) go through the "beta" path (DMA partition-
# broadcast of x1T rows + SBUF bf16 VectorE mult + per-j T-stationary
# matmuls accumulating the j-sum in PSUM — no per-element PSUM toll);
# j in [NJB, 128) go through the "alpha" path (stage-1 matmul + per-j
# scale on DVE/ScalarE + identity-matmul reduce) to soak up ScalarE.
NJB = 96              # beta j's
NJA = 128 - NJB       # alpha j's
NUA = NJA // 4        # alpha 4-j units per tile
NGB = NJB // 4        # beta 4-j groups per 512-row block
NB = BH // 512        # 512-row blocks per core
# alpha units handled by ScalarE activation-scale (rest: wide DVE mult)
ACT_UNITS_A = frozenset({1, 2, 4, 5, 7})

_CACHE = {}
LAST_RESULT = None


def _split_sync_waits(bir_bytes):
    """The walrus build in this container supports exactly ONE sync-wait per
    instruction; Tile freely emits more. Hoist extra waits onto same-engine
    Nop instructions inserted immediately before the owner (engine streams
    are in-order, so 'wait then instruction' == 'instruction with wait').
    Extra completion-updates (non-DMA only) are hoisted onto following Nops.
    """
    import json

    bir = json.loads(bir_bytes)
    ctr = 0
    for fn in bir.get("functions", []):
        for blk in fn.get("blocks", []):
            ins_list = blk.get("instructions")
            if not ins_list:
                continue
            out = []
            for ins in ins_list:
                si = ins.get("sync_info")
                pre, post = [], []
                if si:
                    waits = si.get("on_wait") or []
                    if len(waits) > 1:
                        for w in waits[:-1]:
                            ctr += 1
                            pre.append({
                                "name": f"WSPLIT-{ctr}",
                                "opcode": "NoOp",
                                "engine": ins["engine"],
                                "debug": ins.get("debug", 0),
                                "ins": [],
                                "outs": [],
                                "sync_info": {"on_update": [], "on_wait": [w]},
                            })
                        si["on_wait"] = [waits[-1]]
                    ups = si.get("on_update") or []
                    if len(ups) > 1:
                        if ins.get("opcode") == "DMACopy":
                            raise RuntimeError(
                                f"DMACopy {ins['name']} has {len(ups)} updates; "
                                "cannot hoist safely")
                        for u in ups[1:]:
                            ctr += 1
                            post.append({
                                "name": f"USPLIT-{ctr}",
                                "opcode": "NoOp",
                                "engine": ins["engine"],
                                "debug": ins.get("debug", 0),
                                "ins": [],
                                "outs": [],
                                "sync_info": {"on_update": [u], "on_wait": []},
                            })
                        si["on_update"] = ups[:1]
                out.extend(pre)
                out.append(ins)
                out.extend(post)
            blk["instructions"] = out
    return json.dumps(bir).encode()


def _dedup_ldweights(bir_bytes):
    """Drop PE Ldweights whose operand payload matches the previous weight
    load with only Matmults in between (plain matmuls don't clobber the
    stationary array). Saves ~140 ns of PE time per reload; a tile's 32
    chunk-matmuls all share one x0 stationary. Sync info from a dropped
    load is merged onto the following instruction (the paired Matmult);
    _split_sync_waits then legalizes any multi-wait result.
    """
    import json

    bir = json.loads(bir_bytes)
    n_drop = 0
    for fn in bir.get("functions", []):
        for blk in fn.get("blocks", []):
            ins_list = blk.get("instructions")
            if not ins_list:
                continue
            last_load = None
            out = []
            drop_sync = None
            for ins in ins_list:
                if drop_sync is not None and ins.get("engine") == "PE":
                    si = ins.setdefault(
                        "sync_info", {"on_update": [], "on_wait": []})
                    si["on_wait"] = (drop_sync.get("on_wait") or []) + (
                        si.get("on_wait") or [])
                    si["on_update"] = (si.get("on_update") or []) + (
                        drop_sync.get("on_update") or [])
                    drop_sync = None
                if ins.get("opcode") == "Ldweights":
                    key = json.dumps(
                        [ins.get("ins"), ins.get("perf_mode"),
                         ins.get("is_transpose"), ins.get("tile_position")],
                        sort_keys=True)
                    if last_load == key:
                        si = ins.get("sync_info")
                        if si and (si.get("on_wait") or si.get("on_update")):
                            drop_sync = si
                        n_drop += 1
                        continue
                    last_load = key
                elif ins.get("engine") == "PE" and ins.get("opcode") != "Matmult":
                    last_load = None
                out.append(ins)
            assert drop_sync is None, "dropped Ldweights sync had no successor"
            blk["instructions"] = out
    return json.dumps(bir).encode()


def _install_compile_patch():
    """Route every BIR compile through the dedup + sync-split passes."""
    if _CACHE.get("patched"):
        return
    import concourse.bass_utils as bu

    orig = bu.compile_bir_kernel

    def patched(bir_json, tmpdir, neff_name="file.neff"):
        if isinstance(bir_json, str):
            bir_json = bir_json.encode()
        return orig(_split_sync_waits(_dedup_ldweights(bir_json)),
                    tmpdir, neff_name)

    bu.compile_bir_kernel = patched
    try:
        import concourse.bass2jax as b2j

        b2j.compile_bir_kernel = patched
    except ImportError:
        pass
    _CACHE["patched"] = True


def _build():
    import concourse.bass as bass
    import concourse.mybir as mybir
    from concourse.tile import TileContext

    f32 = mybir.dt.float32
    bf16 = mybir.dt.bfloat16
    f16 = mybir.dt.float16
    nc = bass.Bass()

    x0t = nc.dram_tensor("x0t", [P, BH], bf16, kind="ExternalInput")
    x1t = nc.dram_tensor("x1t", [P, BH], bf16, kind="ExternalInput")
    x1n = nc.dram_tensor("x1n", [BH, P], f32, kind="ExternalInput")
    tcm = nc.dram_tensor("tcm", [P, 128 * 128], bf16, kind="ExternalInput")
    tuv = nc.dram_tensor("tuv", [P, 256], bf16, kind="ExternalInput")
    tcc = nc.dram_tensor("tcc", [1, P], bf16, kind="ExternalInput")
    out = nc.dram_tensor("out", [BH, P], f32, kind="ExternalOutput")

    mult = mybir.AluOpType.mult
    add = mybir.AluOpType.add

    def grp3(ap, n_grp, inner):
        # [P, n_grp*inner] contiguous 2D AP -> 3D [P, n_grp, inner]
        return bass.AP(ap.tensor, ap.offset,
                       [list(ap.ap[0]), [inner, n_grp], [1, inner]])

    def rep_inner(ap, n_grp, n_rep):
        # [P, n_grp] 2D AP -> 3D [P, n_grp, n_rep] with innermost stride 0
        return bass.AP(ap.tensor, ap.offset,
                       [list(ap.ap[0]), [1, n_grp], [0, n_rep]])

    idn = nc.dram_tensor("idn", [P, P], bf16, kind="ExternalInput")

    with TileContext(nc) as tc:
        with (
            tc.tile_pool(name="const", bufs=1) as cpool,
            tc.tile_pool(name="rpsum", bufs=2, space="PSUM") as ppool,
            tc.tile_pool(name="apsum", bufs=2, space="PSUM") as accpool,
            tc.tile_pool(name="bpsum", bufs=1, space="PSUM") as acc2pool,
            tc.tile_pool(name="bcx", bufs=3) as bcxpool,
            tc.tile_pool(name="xb", bufs=2) as xpool,
            tc.tile_pool(name="zbuf", bufs=2) as zpool,
            tc.tile_pool(name="bev", bufs=2) as bevpool,
            tc.tile_pool(name="outb", bufs=2) as opool,
            tc.tile_pool(name="outt", bufs=16) as otpool,
        ):
            # --- resident constants (order = first-consumption order) ---
            # x0t/x1t split into 4 DMAs so the first beta mult isn't gated
            # on one 512 KiB serial transfer
            x0t_s = cpool.tile([P, BH], bf16, tag="x0t")
            for q in range(4):
                nc.sync.dma_start(x0t_s[:, q * 512 : (q + 1) * 512],
                                  x0t[:, q * 512 : (q + 1) * 512])
            x1t_s = cpool.tile([P, BH], bf16, tag="x1t")
            for q in range(4):
                nc.sync.dma_start(x1t_s[:, q * 512 : (q + 1) * 512],
                                  x1t[:, q * 512 : (q + 1) * 512])
            # first two broadcast groups issue before the resident bulk
            # so the beta stream starts immediately
            bcx_tiles = {}

            def issue_bcx(jg):
                # broadcast rows 4jg..4jg+3 (16 KiB contiguous in HBM) to
                # all 128 partitions: one descriptor per partition, split
                # into 8 partition-range DMAs so 8 queues run in parallel
                j0 = 4 * jg
                bcx = bcxpool.tile([P, 4 * BH], bf16, tag="bcx",
                                   name=f"bcx_{jg}")
                srcr = x1t[j0 : j0 + 4, :]
                for h in range(8):
                    nc.sync.dma_start(
                        bcx[h * 16 : (h + 1) * 16, :],
                        bass.AP(srcr.tensor, srcr.offset,
                                [[0, 16], [1, 4 * BH]]))
                bcx_tiles[jg] = bcx

            issue_bcx(0)
            issue_bcx(1)
            tuv_s = cpool.tile([P, 256], bf16, tag="tuv")
            nc.sync.dma_start(tuv_s, tuv[:, :])
            tcc_s = cpool.tile([1, P], bf16, tag="tcc")
            nc.sync.dma_start(tcc_s, tcc[:, :])
            idn_s = cpool.tile([P, P], bf16, tag="idn")
            nc.sync.dma_start(idn_s, idn[:, :])
            ones_s = cpool.tile([1, P], bf16, tag="ones")
            nc.vector.memset(ones_s, 1.0)
            tcm_tiles = []
            for g in range(16):
                tg = cpool.tile([P, 1024], bf16, tag=f"tcm{g}",
                                name=f"tcm_t{g}")
                tcm_tiles.append(tg)
            # alpha's T tiles (12-15) + beta's first tiles load first so
            # neither stream stalls at kernel start
            for g in (12, 13, 14, 15, 0, 1, 2):
                nc.sync.dma_start(tcm_tiles[g],
                                  tcm[:, g * 1024 : (g + 1) * 1024])
            x1n_all = cpool.tile([P, NT * P], f32, tag="x1n_all")
            x1n_full = x1n[:, :]
            for q in range(4):
                nc.sync.dma_start(
                    grp3(x1n_all[:, q * 4 * P : (q + 1) * 4 * P], 4, P),
                    bass.AP(x1n_full.tensor,
                            x1n_full.offset + q * 4 * P * P,
                            [[P, P], [P * P, 4], [1, P]]))
            x1b_all = cpool.tile([P, NT * P], bf16, tag="x1b_all")
            nc.vector.tensor_copy(x1b_all, x1n_all)
            for g in range(3, 12):
                nc.sync.dma_start(tcm_tiles[g],
                                  tcm[:, g * 1024 : (g + 1) * 1024])

            pending = []
            out_ts = {}

            def pairs(ap_base):
                # [P, >=384] AP -> 3D [P, 2, 128] picking 128-col blocks at
                # +0 and +256 (interleaved quarters of the id-matmul acc)
                return bass.AP(ap_base.tensor, ap_base.offset,
                               [list(ap_base.ap[0]), [256, 2], [1, P]])

            # ================= beta path (j in [0, NJB)) =================
            # psum_b[o, b-block] += sum_i T[i,j,o] * (x0T (.) bcast(x1T[j]))
            # j-outer: one full-width broadcast DMA (4 KiB/partition lines),
            # one wide 2x-mode VectorE mult, 4 matmuls per stationary load.
            # The j-sum accumulates in 4 persistent PSUM banks; DVE/ScalarE
            # never touch the 34M-element intermediate.
            acc2 = acc2pool.tile([P, NB * 512], f32, tag="acc2")

            def beta_step(jg):
                if jg + 2 < NJB // 4:
                    issue_bcx(jg + 2)
                bcx = bcx_tiles.pop(jg)
                x4 = xpool.tile([P, 4 * BH], bf16, tag="x4", name=f"x4_{jg}")
                x0full = x0t_s[:, :]
                x0rep = bass.AP(x0full.tensor, x0full.offset,
                                [list(x0full.ap[0]), [0, 4], [1, BH]])
                nc.vector.tensor_tensor(grp3(x4[:, :], 4, BH), x0rep,
                                        grp3(bcx[:, :], 4, BH), mult)
                for jj in range(4):
                    j = 4 * jg + jj
                    for B in range(NB):
                        nc.tensor.matmul(
                            acc2[:, B * 512 : (B + 1) * 512],
                            tcm_tiles[j // 8][
                                :, (j % 8) * 128 : (j % 8) * 128 + 128],
                            x4[:, jj * BH + B * 512 : jj * BH + (B + 1) * 512],
                            start=(j == 0), stop=(j == NJB - 1),
                        )

            # ================= alpha path (j in [NJB, 128)) ==============
            def alpha_phases(t):
                bsl = slice(t * P, (t + 1) * P)
                x1n_t = x1n_all[:, bsl]
                x1b_t = x1b_all[:, bsl]
                zb = zpool.tile([P, NJA * P], bf16, tag="zbuf",
                                name=f"zbuf_{t}")
                acc = accpool.tile([P, 512], f32, tag="acc", name=f"acc_{t}")
                tmp = opool.tile([P, 2 * P], f32, tag="ctmp",
                                 name=f"ctmp_{t}")
                tmp2 = opool.tile([P, 2 * P], f32, tag="ctmp2",
                                  name=f"ctmp2_{t}")
                out_t = otpool.tile([P, P], f32, tag="out_t",
                                    name=f"out_{t}")
                out_ts[t] = out_t
                phases = []

                def ph_uv():
                    nc.tensor.matmul(acc[:, 0:P], ones_s, tcc_s,
                                     start=True, stop=False)
                    nc.tensor.matmul(acc[:, 0:P], x1t_s[:, bsl],
                                     tuv_s[:, 128:256], start=False,
                                     stop=False)
                    nc.tensor.matmul(acc[:, 0:P], x0t_s[:, bsl],
                                     tuv_s[:, 0:128], start=False, stop=False)
                phases.append(ph_uv)

                def ph_unit(u):
                    # 4-j unit: one 512-wide stage-1 matmul + per-j scale
                    def f():
                        r = ppool.tile([P, 512], f32, tag="r",
                                       name=f"r_{t}_{u}")
                        col0 = NJB * P + u * 512
                        nc.tensor.matmul(
                            r, x0t_s[:, bsl],
                            tcm_tiles[col0 // 1024][
                                :, col0 % 1024 : col0 % 1024 + 512],
                            start=True, stop=True,
                        )
                        j0 = 4 * u
                        if u in ACT_UNITS_A:
                            for jj in range(4):
                                nc.scalar.activation(
                                    zb[:, (j0 + jj) * P : (j0 + jj + 1) * P],
                                    r[:, jj * P : (jj + 1) * P],
                                    mybir.ActivationFunctionType.Copy,
                                    scale=x1n_t[:, NJB + j0 + jj
                                                : NJB + j0 + jj + 1],
                                )
                        else:
                            nc.vector.tensor_tensor(
                                grp3(zb[:, j0 * P : (j0 + 4) * P], 4, P),
                                grp3(r[:, :], 4, P),
                                rep_inner(x1b_t[:, NJB + j0 : NJB + j0 + 4],
                                          4, P),
                                mult,
                            )
                    return f
                for u in range(NUA):
                    phases.append(ph_unit(u))

                def ph_ids():
                    for g in range(NJA // 4):
                        nc.tensor.matmul(
                            acc[:, 0:512], idn_s,
                            zb[:, g * 512 : (g + 1) * 512],
                            start=False, stop=(g == NJA // 4 - 1),
                        )
                phases.append(ph_ids)

                def ph_c13():
                    nc.scalar.activation(
                        grp3(tmp[:, :], 2, P), pairs(acc[:, P : 4 * P]),
                        mybir.ActivationFunctionType.Copy)

                def ph_c02():
                    nc.vector.tensor_tensor(
                        grp3(tmp2[:, :], 2, P), pairs(acc[:, 0 : 3 * P]),
                        grp3(tmp[:, :], 2, P), add)

                def ph_cfa():
                    nc.vector.tensor_add(out_t, tmp2[:, 0:P],
                                         tmp2[:, P : 2 * P])
                phases.append(ph_c13)
                phases.append(ph_c02)
                phases.append(ph_cfa)
                return phases

            # interleave the beta j-stream with the alpha tile phases
            aops = []
            for t in range(NT):
                aops += alpha_phases(t)
            # interleave alpha phases into the beta j-pair stream, leaving
            # ~16 alpha phases to trail so they overlap the beta-evac chain
            na, nb_ = len(aops), NJB // 4
            ai = 0
            # head: give the PE queue alpha work (uv + stage-1) so it isn't
            # head-of-line blocked on the first broadcast DMA
            while ai < 10:
                aops[ai]()
                ai += 1
            for jg in range(nb_):
                beta_step(jg)
                target = 10 + (jg + 1) * max(na - 10 - 10, 1) // nb_
                while ai < target:
                    aops[ai]()
                    ai += 1

            # beta evacuation first (ScalarE copy + xbar transpose run while
            # the trailing alpha phases execute), adds + output DMAs last
            tbs = []
            for B in range(NB):
                ob = bevpool.tile([P, 512], bf16, tag="bev", name=f"bev_{B}")
                tb = bevpool.tile([P, 512], bf16, tag="tbt", name=f"tbt_{B}")
                nc.scalar.activation(ob, acc2[:, B * 512 : (B + 1) * 512],
                                     mybir.ActivationFunctionType.Copy)
                for tt in range(4):
                    nc.sync.dma_start(tb[:, tt * P : (tt + 1) * P],
                                      ob[:, tt * P : (tt + 1) * P],
                                      transpose=True)
                tbs.append(tb)
            while ai < na:
                aops[ai]()
                ai += 1
            for B in range(NB):
                for tt in range(4):
                    t = 4 * B + tt
                    nc.vector.tensor_add(
                        out_ts[t], out_ts[t],
                        tbs[B][:, tt * P : (tt + 1) * P])
                    nc.sync.dma_start(out[t * P : (t + 1) * P, :], out_ts[t])

    return nc



def _get_nc():
    if "nc" not in _CACHE:
        _CACHE["nc"] = _build()
    return _CACHE["nc"]


def _make_runner(nc):
    """Persistent sharded-jit runner for the axon/PJRT path (specialized copy
    of bass2jax.run_bass_via_pjrt so repeated calls reuse one compiled
    executable). Returns run(in_maps) -> list[dict[str, np.ndarray]]."""
    import jax
    import numpy as jnp_np  # noqa
    from jax.sharding import Mesh, PartitionSpec
    from jax.experimental.shard_map import shard_map
    import concourse.mybir as mybir
    from concourse.bass2jax import (
        _bass_exec_p, install_neuronx_cc_hook, partition_id_tensor)

    install_neuronx_cc_hook()

    partition_name = nc.partition_id_tensor.name if nc.partition_id_tensor else None
    in_names, out_names, out_avals, zero_outs = [], [], [], []
    for alloc in nc.m.functions[0].allocations:
        if not isinstance(alloc, mybir.MemoryLocationSet):
            continue
        name = alloc.memorylocations[0].name
        if alloc.kind == "ExternalInput":
            if name != partition_name:
                in_names.append(name)
        elif alloc.kind == "ExternalOutput":
            out_names.append(name)
            shape = tuple(alloc.tensor_shape)
            dtype = mybir.dt.np(alloc.dtype)
            out_avals.append(jax.core.ShapedArray(shape, dtype))
            zero_outs.append(np.zeros(shape, dtype))
    n_params = len(in_names)
    n_outs = len(out_avals)
    all_in_names = list(in_names) + list(out_names)
    if partition_name is not None:
        all_in_names.append(partition_name)
    donate = tuple(range(n_params, n_params + n_outs))

    def _body(*args):
        operands = list(args)
        if partition_name is not None:
            operands.append(partition_id_tensor())
        outs = _bass_exec_p.bind(
            *operands,
            out_avals=tuple(out_avals),
            in_names=tuple(all_in_names),
            out_names=tuple(out_names),
            lowering_input_output_aliases=(),
            sim_require_finite=True,
            sim_require_nnan=True,
            nc=nc,
        )
        return tuple(outs)

    devices = jax.devices()[:NCORES]
    mesh = Mesh(np.asarray(devices), ("core",))
    in_specs = (PartitionSpec("core"),) * (n_params + n_outs)
    out_specs = (PartitionSpec("core"),) * len(out_names)
    sharded = jax.jit(
        shard_map(_body, mesh=mesh, in_specs=in_specs, out_specs=out_specs,
                  check_rep=False),
        donate_argnums=donate, keep_unused=True)

    def run(in_maps, raw=False):
        concat_in = [
            np.concatenate([np.asarray(m[name]) for m in in_maps], axis=0)
            for name in in_names
        ]
        concat_zeros = [
            np.zeros((NCORES * z.shape[0], *z.shape[1:]), z.dtype)
            for z in zero_outs
        ]
        out_arrs = sharded(*concat_in, *concat_zeros)
        if raw:
            return out_arrs
        return [
            {name: np.asarray(out_arrs[i]).reshape(NCORES, *out_avals[i].shape)[c]
             for i, name in enumerate(out_names)}
            for c in range(NCORES)
        ]

    return run


def _run(nc, in_maps):
    """Execute on 8 cores; under axon go through the persistent PJRT runner."""
    from concourse._compat import axon_active

    _install_compile_patch()

    if axon_active():
        if "runner" not in _CACHE:
            _CACHE["runner"] = _make_runner(nc)
        return _CACHE["runner"](in_maps), None

    from concourse.bass_utils import run_bass_kernel_spmd

    res = run_bass_kernel_spmd(nc, in_maps, core_ids=list(range(NCORES)))
    return res.results, res


def _make_in_maps(x0, x1, T):
    import ml_dtypes

    bf16 = ml_dtypes.bfloat16
    x0 = np.asarray(x0, dtype=np.float32)
    x1 = np.asarray(x1, dtype=np.float32)
    T = np.asarray(T, dtype=np.float32)

    idn = np.eye(P, dtype=np.float32).astype(bf16)
    in_maps = []
    for c in range(NCORES):
        a, h = divmod(c, 2)
        bsl = slice(h * BH, (h + 1) * BH)
        x0c = np.ascontiguousarray(x0[bsl, a, :])  # (BH, 128)
        x1c = np.ascontiguousarray(x1[bsl, a, :])
        in_maps.append({
            "x0t": np.ascontiguousarray(x0c.T).astype(bf16),
            "x1t": np.ascontiguousarray(x1c.T).astype(bf16),
            "x1n": x1c,
            "tcm": np.ascontiguousarray(
                T[a, :128, :128, :].reshape(128, 128 * 128)).astype(bf16),
            "tuv": np.ascontiguousarray(
                np.concatenate([T[a, :128, 128, :], T[a, 128, :128, :]],
                               axis=1)).astype(bf16),
            "tcc": np.ascontiguousarray(
                T[a, 128, 128, :].reshape(1, 128)).astype(bf16),
            "idn": idn,
        })
    return in_maps


def kernel(x0, x1, T):
    global LAST_RESULT

    in_maps = _make_in_maps(x0, x1, T)
    nc = _get_nc()
    results, LAST_RESULT = _run(nc, in_maps)

    full = np.empty((BS, A, OUT), dtype=np.float32)
    for c in range(NCORES):
        a, h = divmod(c, 2)
        full[h * BH : (h + 1) * BH, a, :] = results[c]["out"]
    return full



# revision 20
# speedup vs baseline: 2.1764x; 2.1764x over previous
"""Trainium2 Bass kernel for nn_AugmentedTensor (per-head bilinear form).

out[b,a,o] = sum_{i,j} h0[b,a,i] * h1[b,a,j] * T[a,i,j,o],  h = concat(x, 1)

Decomposition (i<128 from x0, i=128 is the ones row; same for j):
  main = sum_{i<128,j<128} x0[b,i] x1[b,j] T[a,i,j,o]
       -> stage1 (PE):  r[b,(j,o)] = x0_tile @ Tc[a]          (K=128, N=16384)
       -> stage2a (DVE+ACT): z_j[b,o] = x1[b,j]*r[b,j,o] -> bf16 zbuf
       -> stage2b (PE): identity-stationary matmuls accumulate the 128
          z slices into one PSUM bank as 4 interleaved quarters
          (psum[:, q*128+o] += z_{4r+q}), 4 slices per N=512 matmul.
          This moves the whole j-reduction tree off VectorE onto TensorE
          PSUM accumulation and keeps the PE dense (HAM stays warm).
  uv   = x0 @ T[a,:128,128,:] + x1 @ T[a,128,:128,:] + T[a,128,128,:]
         accumulated directly into quarter 0 of the same PSUM bank.
  final: out = q0+q1+q2+q3 (VectorE, 3 adds) -> DMA.

Sharding: 8 cores; core c -> head a=c>>1, batch half c&1 (2048 rows).
T traffic per core = one head (~8.5 MB) instead of 34 MB replicated.
"""

import numpy as np

BS, A, D, OUT = 4096, 4, 128, 128
NCORES = 8
BH = BS // 2      # batch rows per core
P = 128
NT = BH // P      # 16 tiles per core
NJ = 128          # j slices per tile
NCH = 32          # 512-wide stage-1 chunks per tile

# Hybrid j-split: j in [0, NJB) go through the "beta" path (DMA partition-
# broadcast of x1T rows + SBUF bf16 VectorE mult + per-j T-stationary
# matmuls accumulating the j-sum in PSUM — no per-element PSUM toll);
# j in [NJB, 128) go through the "alpha" path (stage-1 matmul + per-j
# scale on DVE/ScalarE + identity-matmul reduce) to soak up ScalarE.
NJB = 96              # beta j's
NJA = 128 - NJB       # alpha j's
NUA = NJA // 4        # alpha 4-j units per tile
NGB = NJB // 4        # beta 4-j groups per 512-row block
NB = BH // 512        # 512-row blocks per core
# alpha units handled by ScalarE activation-scale (rest: wide DVE mult)
ACT_UNITS_A = frozenset({1, 2, 4, 5, 7})

_CACHE = {}
LAST_RESULT = None


def _split_sync_waits(bir_bytes):
    """The walrus build in this container supports exactly ONE sync-wait per
    instruction; Tile freely emits more. Hoist extra waits onto same-engine
    Nop instructions inserted immediately before the owner (engine streams
    are in-order, so 'wait then instruction' == 'instruction with wait').
    Extra completion-updates (non-DMA only) are hoisted onto following Nops.
    """
    import json

    bir = json.loads(bir_bytes)
    ctr = 0
    for fn in bir.get("functions", []):
        for blk in fn.get("blocks", []):
            ins_list = blk.get("instructions")
            if not ins_list:
                continue
            out = []
            for ins in ins_list:
                si = ins.get("sync_info")
                pre, post = [], []
                if si:
                    waits = si.get("on_wait") or []
                    if len(waits) > 1:
                        for w in waits[:-1]:
                            ctr += 1
                            pre.append({
                                "name": f"WSPLIT-{ctr}",
                                "opcode": "NoOp",
                                "engine": ins["engine"],
                                "debug": ins.get("debug", 0),
                                "ins": [],
                                "outs": [],
                                "sync_info": {"on_update": [], "on_wait": [w]},
                            })
                        si["on_wait"] = [waits[-1]]
                    ups = si.get("on_update") or []
                    if len(ups) > 1:
                        if ins.get("opcode") == "DMACopy":
                            raise RuntimeError(
                                f"DMACopy {ins['name']} has {len(ups)} updates; "
                                "cannot hoist safely")
                        for u in ups[1:]:
                            ctr += 1
                            post.append({
                                "name": f"USPLIT-{ctr}",
                                "opcode": "NoOp",
                                "engine": ins["engine"],
                                "debug": ins.get("debug", 0),
                                "ins": [],
                                "outs": [],
                                "sync_info": {"on_update": [u], "on_wait": []},
                            })
                        si["on_update"] = ups[:1]
                out.extend(pre)
                out.append(ins)
                out.extend(post)
            blk["instructions"] = out
    return json.dumps(bir).encode()


def _dedup_ldweights(bir_bytes):
    """Drop PE Ldweights whose operand payload matches the previous weight
    load with only Matmults in between (plain matmuls don't clobber the
    stationary array). Saves ~140 ns of PE time per reload; a tile's 32
    chunk-matmuls all share one x0 stationary. Sync info from a dropped
    load is merged onto the following instruction (the paired Matmult);
    _split_sync_waits then legalizes any multi-wait result.
    """
    import json

    bir = json.loads(bir_bytes)
    n_drop = 0
    for fn in bir.get("functions", []):
        for blk in fn.get("blocks", []):
            ins_list = blk.get("instructions")
            if not ins_list:
                continue
            last_load = None
            out = []
            drop_sync = None
            for ins in ins_list:
                if drop_sync is not None and ins.get("engine") == "PE":
                    si = ins.setdefault(
                        "sync_info", {"on_update": [], "on_wait": []})
                    si["on_wait"] = (drop_sync.get("on_wait") or []) + (
                        si.get("on_wait") or [])
                    si["on_update"] = (si.get("on_update") or []) + (
                        drop_sync.get("on_update") or [])
                    drop_sync = None
                if ins.get("opcode") == "Ldweights":
                    key = json.dumps(
                        [ins.get("ins"), ins.get("perf_mode"),
                         ins.get("is_transpose"), ins.get("tile_position")],
                        sort_keys=True)
                    if last_load == key:
                        si = ins.get("sync_info")
                        if si and (si.get("on_wait") or si.get("on_update")):
                            drop_sync = si
                        n_drop += 1
                        continue
                    last_load = key
                elif ins.get("engine") == "PE" and ins.get("opcode") != "Matmult":
                    last_load = None
                out.append(ins)
            assert drop_sync is None, "dropped Ldweights sync had no successor"
            blk["instructions"] = out
    return json.dumps(bir).encode()


def _install_compile_patch():
    """Route every BIR compile through the dedup + sync-split passes."""
    if _CACHE.get("patched"):
        return
    import concourse.bass_utils as bu

    orig = bu.compile_bir_kernel

    def patched(bir_json, tmpdir, neff_name="file.neff"):
        if isinstance(bir_json, str):
            bir_json = bir_json.encode()
        return orig(_split_sync_waits(_dedup_ldweights(bir_json)),
                    tmpdir, neff_name)

    bu.compile_bir_kernel = patched
    try:
        import concourse.bass2jax as b2j

        b2j.compile_bir_kernel = patched
    except ImportError:
        pass
    _CACHE["patched"] = True


def _build():
    import concourse.bass as bass
    import concourse.mybir as mybir
    from concourse.tile import TileContext

    f32 = mybir.dt.float32
    bf16 = mybir.dt.bfloat16
    f16 = mybir.dt.float16
    nc = bass.Bass()

    x0t = nc.dram_tensor("x0t", [P, BH], bf16, kind="ExternalInput")
    x1t = nc.dram_tensor("x1t", [P, BH], bf16, kind="ExternalInput")
    x1n = nc.dram_tensor("x1n", [BH, P], f32, kind="ExternalInput")
    tcm = nc.dram_tensor("tcm", [P, 128 * 128], bf16, kind="ExternalInput")
    tuv = nc.dram_tensor("tuv", [P, 256], bf16, kind="ExternalInput")
    tcc = nc.dram_tensor("tcc", [1, P], bf16, kind="ExternalInput")
    out = nc.dram_tensor("out", [BH, P], f32, kind="ExternalOutput")

    mult = mybir.AluOpType.mult
    add = mybir.AluOpType.add

    def grp3(ap, n_grp, inner):
        # [P, n_grp*inner] contiguous 2D AP -> 3D [P, n_grp, inner]
        return bass.AP(ap.tensor, ap.offset,
                       [list(ap.ap[0]), [inner, n_grp], [1, inner]])

    def rep_inner(ap, n_grp, n_rep):
        # [P, n_grp] 2D AP -> 3D [P, n_grp, n_rep] with innermost stride 0
        return bass.AP(ap.tensor, ap.offset,
                       [list(ap.ap[0]), [1, n_grp], [0, n_rep]])

    idn = nc.dram_tensor("idn", [P, P], bf16, kind="ExternalInput")

    with TileContext(nc) as tc:
        with (
            tc.tile_pool(name="const", bufs=1) as cpool,
            tc.tile_pool(name="rpsum", bufs=2, space="PSUM") as ppool,
            tc.tile_pool(name="apsum", bufs=2, space="PSUM") as accpool,
            tc.tile_pool(name="bpsum", bufs=1, space="PSUM") as acc2pool,
            tc.tile_pool(name="bcx", bufs=3) as bcxpool,
            tc.tile_pool(name="xb", bufs=2) as xpool,
            tc.tile_pool(name="zbuf", bufs=2) as zpool,
            tc.tile_pool(name="bev", bufs=2) as bevpool,
            tc.tile_pool(name="outb", bufs=2) as opool,
            tc.tile_pool(name="outt", bufs=16) as otpool,
        ):
            # --- resident constants (order = first-consumption order) ---
            # x0t/x1t split into 4 DMAs so the first beta mult isn't gated
            # on one 512 KiB serial transfer
            x0t_s = cpool.tile([P, BH], bf16, tag="x0t")
            for q in range(4):
                nc.sync.dma_start(x0t_s[:, q * 512 : (q + 1) * 512],
                                  x0t[:, q * 512 : (q + 1) * 512])
            x1t_s = cpool.tile([P, BH], bf16, tag="x1t")
            for q in range(4):
                nc.sync.dma_start(x1t_s[:, q * 512 : (q + 1) * 512],
                                  x1t[:, q * 512 : (q + 1) * 512])
            # first two broadcast groups issue before the resident bulk
            # so the beta stream starts immediately
            bcx_tiles = {}

            def issue_bcx(jg):
                # broadcast rows 4jg..4jg+3 (16 KiB contiguous in HBM) to
                # all 128 partitions: one descriptor per partition, split
                # into 8 partition-range DMAs so 8 queues run in parallel
                j0 = 4 * jg
                bcx = bcxpool.tile([P, 4 * BH], bf16, tag="bcx",
                                   name=f"bcx_{jg}")
                srcr = x1t[j0 : j0 + 4, :]
                nc.sync.dma_start(
                    bcx, bass.AP(srcr.tensor, srcr.offset,
                                 [[0, P], [1, 4 * BH]]))
                bcx_tiles[jg] = bcx

            issue_bcx(0)
            issue_bcx(1)
            tuv_s = cpool.tile([P, 256], bf16, tag="tuv")
            nc.sync.dma_start(tuv_s, tuv[:, :])
            tcc_s = cpool.tile([1, P], bf16, tag="tcc")
            nc.sync.dma_start(tcc_s, tcc[:, :])
            idn_s = cpool.tile([P, P], bf16, tag="idn")
            nc.sync.dma_start(idn_s, idn[:, :])
            ones_s = cpool.tile([1, P], bf16, tag="ones")
            nc.vector.memset(ones_s, 1.0)
            tcm_tiles = []
            for g in range(16):
                tg = cpool.tile([P, 1024], bf16, tag=f"tcm{g}",
                                name=f"tcm_t{g}")
                tcm_tiles.append(tg)
            # alpha's T tiles (12-15) + beta's first tiles load first so
            # neither stream stalls at kernel start
            for g in (12, 13, 14, 15, 0, 1, 2):
                nc.sync.dma_start(tcm_tiles[g],
                                  tcm[:, g * 1024 : (g + 1) * 1024])
            x1n_all = cpool.tile([P, NT * P], f32, tag="x1n_all")
            x1n_full = x1n[:, :]
            for q in range(4):
                nc.sync.dma_start(
                    grp3(x1n_all[:, q * 4 * P : (q + 1) * 4 * P], 4, P),
                    bass.AP(x1n_full.tensor,
                            x1n_full.offset + q * 4 * P * P,
                            [[P, P], [P * P, 4], [1, P]]))
            x1b_all = cpool.tile([P, NT * P], bf16, tag="x1b_all")
            nc.vector.tensor_copy(x1b_all, x1n_all)
            for g in range(3, 12):
                nc.sync.dma_start(tcm_tiles[g],
                                  tcm[:, g * 1024 : (g + 1) * 1024])

            pending = []
            out_ts = {}

            def pairs(ap_base):
                # [P, >=384] AP -> 3D [P, 2, 128] picking 128-col blocks at
                # +0 and +256 (interleaved quarters of the id-matmul acc)
                return bass.AP(ap_base.tensor, ap_base.offset,
                               [list(ap_base.ap[0]), [256, 2], [1, P]])

            # ================= beta path (j in [0, NJB)) =================
            # psum_b[o, b-block] += sum_i T[i,j,o] * (x0T (.) bcast(x1T[j]))
            # j-outer: one full-width broadcast DMA (4 KiB/partition lines),
            # one wide 2x-mode VectorE mult, 4 matmuls per stationary load.
            # The j-sum accumulates in 4 persistent PSUM banks; DVE/ScalarE
            # never touch the 34M-element intermediate.
            acc2 = acc2pool.tile([P, NB * 512], f32, tag="acc2")

            def beta_step(jg):
                if jg + 2 < NJB // 4:
                    issue_bcx(jg + 2)
                bcx = bcx_tiles.pop(jg)
                x4 = xpool.tile([P, 4 * BH], bf16, tag="x4", name=f"x4_{jg}")
                x0full = x0t_s[:, :]
                x0rep = bass.AP(x0full.tensor, x0full.offset,
                                [list(x0full.ap[0]), [0, 4], [1, BH]])
                nc.vector.tensor_tensor(grp3(x4[:, :], 4, BH), x0rep,
                                        grp3(bcx[:, :], 4, BH), mult)
                for jj in range(4):
                    j = 4 * jg + jj
                    for B in range(NB):
                        nc.tensor.matmul(
                            acc2[:, B * 512 : (B + 1) * 512],
                            tcm_tiles[j // 8][
                                :, (j % 8) * 128 : (j % 8) * 128 + 128],
                            x4[:, jj * BH + B * 512 : jj * BH + (B + 1) * 512],
                            start=(j == 0), stop=(j == NJB - 1),
                        )

            # ================= alpha path (j in [NJB, 128)) ==============
            def alpha_phases(t):
                bsl = slice(t * P, (t + 1) * P)
                x1n_t = x1n_all[:, bsl]
                x1b_t = x1b_all[:, bsl]
                zb = zpool.tile([P, NJA * P], bf16, tag="zbuf",
                                name=f"zbuf_{t}")
                acc = accpool.tile([P, 512], f32, tag="acc", name=f"acc_{t}")
                tmp = opool.tile([P, 2 * P], f32, tag="ctmp",
                                 name=f"ctmp_{t}")
                tmp2 = opool.tile([P, 2 * P], f32, tag="ctmp2",
                                  name=f"ctmp2_{t}")
                out_t = otpool.tile([P, P], f32, tag="out_t",
                                    name=f"out_{t}")
                out_ts[t] = out_t
                phases = []

                def ph_uv():
                    nc.tensor.matmul(acc[:, 0:P], ones_s, tcc_s,
                                     start=True, stop=False)
                    nc.tensor.matmul(acc[:, 0:P], x1t_s[:, bsl],
                                     tuv_s[:, 128:256], start=False,
                                     stop=False)
                    nc.tensor.matmul(acc[:, 0:P], x0t_s[:, bsl],
                                     tuv_s[:, 0:128], start=False, stop=False)
                phases.append(ph_uv)

                def ph_unit(u):
                    # 4-j unit: one 512-wide stage-1 matmul + per-j scale
                    def f():
                        r = ppool.tile([P, 512], f32, tag="r",
                                       name=f"r_{t}_{u}")
                        col0 = NJB * P + u * 512
                        nc.tensor.matmul(
                            r, x0t_s[:, bsl],
                            tcm_tiles[col0 // 1024][
                                :, col0 % 1024 : col0 % 1024 + 512],
                            start=True, stop=True,
                        )
                        j0 = 4 * u
                        if u in ACT_UNITS_A:
                            for jj in range(4):
                                nc.scalar.activation(
                                    zb[:, (j0 + jj) * P : (j0 + jj + 1) * P],
                                    r[:, jj * P : (jj + 1) * P],
                                    mybir.ActivationFunctionType.Copy,
                                    scale=x1n_t[:, NJB + j0 + jj
                                                : NJB + j0 + jj + 1],
                                )
                        else:
                            nc.vector.tensor_tensor(
                                grp3(zb[:, j0 * P : (j0 + 4) * P], 4, P),
                                grp3(r[:, :], 4, P),
                                rep_inner(x1b_t[:, NJB + j0 : NJB + j0 + 4],
                                          4, P),
                                mult,
                            )
                    return f
                for u in range(NUA):
                    phases.append(ph_unit(u))

                def ph_ids():
                    for g in range(NJA // 4):
                        nc.tensor.matmul(
                            acc[:, 0:512], idn_s,
                            zb[:, g * 512 : (g + 1) * 512],
                            start=False, stop=(g == NJA // 4 - 1),
                        )
                phases.append(ph_ids)

                def ph_c13():
                    nc.scalar.activation(
                        grp3(tmp[:, :], 2, P), pairs(acc[:, P : 4 * P]),
                        mybir.ActivationFunctionType.Copy)

                def ph_c02():
                    nc.vector.tensor_tensor(
                        grp3(tmp2[:, :], 2, P), pairs(acc[:, 0 : 3 * P]),
                        grp3(tmp[:, :], 2, P), add)

                def ph_cfa():
                    nc.vector.tensor_add(out_t, tmp2[:, 0:P],
                                         tmp2[:, P : 2 * P])
                phases.append(ph_c13)
                phases.append(ph_c02)
                phases.append(ph_cfa)
                return phases

            # interleave the beta j-stream with the alpha tile phases
            aops = []
            for t in range(NT):
                aops += alpha_phases(t)
            # interleave alpha phases into the beta j-pair stream, leaving
            # ~16 alpha phases to trail so they overlap the beta-evac chain
            na, nb_ = len(aops), NJB // 4
            ai = 0
            # head: give the PE queue alpha work (uv + stage-1) so it isn't
            # head-of-line blocked on the first broadcast DMA
            while ai < 10:
                aops[ai]()
                ai += 1
            for jg in range(nb_):
                beta_step(jg)
                target = 10 + (jg + 1) * max(na - 10 - 10, 1) // nb_
                while ai < target:
                    aops[ai]()
                    ai += 1

            # beta evacuation first (ScalarE copy + xbar transpose run while
            # the trailing alpha phases execute), adds + output DMAs last
            tbs = []
            for B in range(NB):
                ob = bevpool.tile([P, 512], bf16, tag="bev", name=f"bev_{B}")
                tb = bevpool.tile([P, 512], bf16, tag="tbt", name=f"tbt_{B}")
                nc.scalar.activation(ob, acc2[:, B * 512 : (B + 1) * 512],
                                     mybir.ActivationFunctionType.Copy)
                for tt in range(4):
                    nc.sync.dma_start(tb[:, tt * P : (tt + 1) * P],
                                      ob[:, tt * P : (tt + 1) * P],
                                      transpose=True)
                tbs.append(tb)
            while ai < na:
                aops[ai]()
                ai += 1
            for B in range(NB):
                for tt in range(4):
                    t = 4 * B + tt
                    nc.vector.tensor_add(
                        out_ts[t], out_ts[t],
                        tbs[B][:, tt * P : (tt + 1) * P])
                    nc.sync.dma_start(out[t * P : (t + 1) * P, :], out_ts[t])

    return nc



def _get_nc():
    if "nc" not in _CACHE:
        _CACHE["nc"] = _build()
    return _CACHE["nc"]


def _make_runner(nc):
    """Persistent sharded-jit runner for the axon/PJRT path (specialized copy
    of bass2jax.run_bass_via_pjrt so repeated calls reuse one compiled
    executable). Returns run(in_maps) -> list[dict[str, np.ndarray]]."""
    import jax
    import numpy as jnp_np  # noqa
    from jax.sharding import Mesh, PartitionSpec
    from jax.experimental.shard_map import shard_map
    import concourse.mybir as mybir
    from concourse.bass2jax import (
        _bass_exec_p, install_neuronx_cc_hook, partition_id_tensor)

    install_neuronx_cc_hook()

    partition_name = nc.partition_id_tensor.name if nc.partition_id_tensor else None
    in_names, out_names, out_avals, zero_outs = [], [], [], []
    for alloc in nc.m.functions[0].allocations:
        if not isinstance(alloc, mybir.MemoryLocationSet):
            continue
        name = alloc.memorylocations[0].name
        if alloc.kind == "ExternalInput":
            if name != partition_name:
                in_names.append(name)
        elif alloc.kind == "ExternalOutput":
            out_names.append(name)
            shape = tuple(alloc.tensor_shape)
            dtype = mybir.dt.np(alloc.dtype)
            out_avals.append(jax.core.ShapedArray(shape, dtype))
            zero_outs.append(np.zeros(shape, dtype))
    n_params = len(in_names)
    n_outs = len(out_avals)
    all_in_names = list(in_names) + list(out_names)
    if partition_name is not None:
        all_in_names.append(partition_name)
    donate = tuple(range(n_params, n_params + n_outs))

    def _body(*args):
        operands = list(args)
        if partition_name is not None:
            operands.append(partition_id_tensor())
        outs = _bass_exec_p.bind(
            *operands,
            out_avals=tuple(out_avals),
            in_names=tuple(all_in_names),
            out_names=tuple(out_names),
            lowering_input_output_aliases=(),
            sim_require_finite=True,
            sim_require_nnan=True,
            nc=nc,
        )
        return tuple(outs)

    devices = jax.devices()[:NCORES]
    mesh = Mesh(np.asarray(devices), ("core",))
    in_specs = (PartitionSpec("core"),) * (n_params + n_outs)
    out_specs = (PartitionSpec("core"),) * len(out_names)
    sharded = jax.jit(
        shard_map(_body, mesh=mesh, in_specs=in_specs, out_specs=out_specs,
                  check_rep=False),
        donate_argnums=donate, keep_unused=True)

    def run(in_maps, raw=False):
        concat_in = [
            np.concatenate([np.asarray(m[name]) for m in in_maps], axis=0)
            for name in in_names
        ]
        concat_zeros = [
            np.zeros((NCORES * z.shape[0], *z.shape[1:]), z.dtype)
            for z in zero_outs
        ]
        out_arrs = sharded(*concat_in, *concat_zeros)
        if raw:
            return out_arrs
        return [
            {name: np.asarray(out_arrs[i]).reshape(NCORES, *out_avals[i].shape)[c]
             for i, name in enumerate(out_names)}
            for c in range(NCORES)
        ]

    return run


def _run(nc, in_maps):
    """Execute on 8 cores; under axon go through the persistent PJRT runner."""
    from concourse._compat import axon_active

    _install_compile_patch()

    if axon_active():
        if "runner" not in _CACHE:
            _CACHE["runner"] = _make_runner(nc)
        return _CACHE["runner"](in_maps), None

    from concourse.bass_utils import run_bass_kernel_spmd

    res = run_bass_kernel_spmd(nc, in_maps, core_ids=list(range(NCORES)))
    return res.results, res


def _make_in_maps(x0, x1, T):
    import ml_dtypes

    bf16 = ml_dtypes.bfloat16
    x0 = np.asarray(x0, dtype=np.float32)
    x1 = np.asarray(x1, dtype=np.float32)
    T = np.asarray(T, dtype=np.float32)

    idn = np.eye(P, dtype=np.float32).astype(bf16)
    in_maps = []
    for c in range(NCORES):
        a, h = divmod(c, 2)
        bsl = slice(h * BH, (h + 1) * BH)
        x0c = np.ascontiguousarray(x0[bsl, a, :])  # (BH, 128)
        x1c = np.ascontiguousarray(x1[bsl, a, :])
        in_maps.append({
            "x0t": np.ascontiguousarray(x0c.T).astype(bf16),
            "x1t": np.ascontiguousarray(x1c.T).astype(bf16),
            "x1n": x1c,
            "tcm": np.ascontiguousarray(
                T[a, :128, :128, :].reshape(128, 128 * 128)).astype(bf16),
            "tuv": np.ascontiguousarray(
                np.concatenate([T[a, :128, 128, :], T[a, 128, :128, :]],
                               axis=1)).astype(bf16),
            "tcc": np.ascontiguousarray(
                T[a, 128, 128, :].reshape(1, 128)).astype(bf16),
            "idn": idn,
        })
    return in_maps


def kernel(x0, x1, T):
    global LAST_RESULT

    in_maps = _make_in_maps(x0, x1, T)
    nc = _get_nc()
    results, LAST_RESULT = _run(nc, in_maps)

    full = np.empty((BS, A, OUT), dtype=np.float32)
    for c in range(NCORES):
        a, h = divmod(c, 2)
        full[h * BH : (h + 1) * BH, a, :] = results[c]["out"]
    return full



# revision 22
# speedup vs baseline: 2.3653x; 1.0868x over previous
"""Trainium2 Bass kernel for nn_AugmentedTensor (per-head bilinear form).

out[b,a,o] = sum_{i,j} h0[b,a,i] * h1[b,a,j] * T[a,i,j,o],  h = concat(x, 1)

Hybrid two-path decomposition of the main term (i<128, j<128):

beta path (j < NJB=96) — j-sum accumulated INSIDE PSUM, so no engine ever
touches the (b x j x o) intermediate elementwise:
  - idle DMA engines broadcast x1T row-pairs (contiguous 8 KiB in HBM) to
    all 128 partitions (one descriptor per partition),
  - VectorE forms X_j[i,b] = x0T (.) bcast(x1T[j]) in SBUF bf16 (2x mode),
  - TensorE accumulates psum[o, b-block] += T[:,j,:]^T @ X_j over all j
    with per-j stationaries (4 persistent PSUM banks, b-blocks of 512).
  - evac: ScalarE copy -> bf16, DMA xbar transpose [o,b]->[b,o], add.

alpha path (j >= NJB) — classic two-stage, exists to soak up ScalarE
(which cannot multiply two tensors and is otherwise idle):
  - stage1 (PE): r[b,(j,o)] chunks = x0_tile @ T chunks (K=128, N=512),
  - per-j scale z_j = x1[b,j]*r[b,j,:] split VectorE wide-mult / ScalarE
    activation-scale -> bf16 zbuf,
  - identity-stationary matmuls accumulate the z slices into one PSUM bank
    as 4 interleaved quarters; uv bias terms accumulate into quarter 0.
  - final: out_t = q0+q1+q2+q3 (+ transposed beta part) -> DMA.

uv   = x0 @ T[a,:128,128,:] + x1 @ T[a,128,:128,:] + T[a,128,128,:]

Sharding: 8 cores; core c -> head a=c>>1, batch half c&1 (2048 rows).
T traffic per core = one head (~4.2 MB bf16) instead of 34 MB replicated.
486 us (staged baseline) -> ~260 us measured on 8 axon trn2 cores.
"""

import numpy as np

BS, A, D, OUT = 4096, 4, 128, 128
NCORES = 8
BH = BS // 2      # batch rows per core
P = 128
NT = BH // P      # 16 tiles per core

# Hybrid j-split: j in [0, NJB) go through the "beta" path (DMA partition-
# broadcast of x1T rows + SBUF bf16 VectorE mult + per-j T-stationary
# matmuls accumulating the j-sum in PSUM — no per-element PSUM toll);
# j in [NJB, 128) go through the "alpha" path (stage-1 matmul + per-j
# scale on DVE/ScalarE + identity-matmul reduce) to soak up ScalarE.
NJB = 96              # beta j's
NJA = 128 - NJB       # alpha j's
NUA = NJA // 4        # alpha 4-j units per tile
NB = BH // 512        # 512-row b-blocks per core (beta matmul N-extent)
# alpha units handled by ScalarE activation-scale (rest: wide DVE mult)
ACT_UNITS_A = frozenset({1, 2, 4, 5, 7})

_CACHE = {}
LAST_RESULT = None


def _split_sync_waits(bir_bytes):
    """The walrus build in this container supports exactly ONE sync-wait per
    instruction; Tile freely emits more. Hoist extra waits onto same-engine
    Nop instructions inserted immediately before the owner (engine streams
    are in-order, so 'wait then instruction' == 'instruction with wait').
    Extra completion-updates (non-DMA only) are hoisted onto following Nops.
    """
    import json

    bir = json.loads(bir_bytes)
    ctr = 0
    for fn in bir.get("functions", []):
        for blk in fn.get("blocks", []):
            ins_list = blk.get("instructions")
            if not ins_list:
                continue
            out = []
            for ins in ins_list:
                si = ins.get("sync_info")
                pre, post = [], []
                if si:
                    waits = si.get("on_wait") or []
                    if len(waits) > 1:
                        for w in waits[:-1]:
                            ctr += 1
                            pre.append({
                                "name": f"WSPLIT-{ctr}",
                                "opcode": "NoOp",
                                "engine": ins["engine"],
                                "debug": ins.get("debug", 0),
                                "ins": [],
                                "outs": [],
                                "sync_info": {"on_update": [], "on_wait": [w]},
                            })
                        si["on_wait"] = [waits[-1]]
                    ups = si.get("on_update") or []
                    if len(ups) > 1:
                        if ins.get("opcode") == "DMACopy":
                            raise RuntimeError(
                                f"DMACopy {ins['name']} has {len(ups)} updates; "
                                "cannot hoist safely")
                        for u in ups[1:]:
                            ctr += 1
                            post.append({
                                "name": f"USPLIT-{ctr}",
                                "opcode": "NoOp",
                                "engine": ins["engine"],
                                "debug": ins.get("debug", 0),
                                "ins": [],
                                "outs": [],
                                "sync_info": {"on_update": [u], "on_wait": []},
                            })
                        si["on_update"] = ups[:1]
                out.extend(pre)
                out.append(ins)
                out.extend(post)
            blk["instructions"] = out
    return json.dumps(bir).encode()


def _dedup_ldweights(bir_bytes):
    """Drop PE Ldweights whose operand payload matches the previous weight
    load with only Matmults in between (plain matmuls don't clobber the
    stationary array). Saves ~140 ns of PE time per reload; a tile's 32
    chunk-matmuls all share one x0 stationary. Sync info from a dropped
    load is merged onto the following instruction (the paired Matmult);
    _split_sync_waits then legalizes any multi-wait result.
    """
    import json

    bir = json.loads(bir_bytes)
    n_drop = 0
    for fn in bir.get("functions", []):
        for blk in fn.get("blocks", []):
            ins_list = blk.get("instructions")
            if not ins_list:
                continue
            last_load = None
            out = []
            drop_sync = None
            for ins in ins_list:
                if drop_sync is not None and ins.get("engine") == "PE":
                    si = ins.setdefault(
                        "sync_info", {"on_update": [], "on_wait": []})
                    si["on_wait"] = (drop_sync.get("on_wait") or []) + (
                        si.get("on_wait") or [])
                    si["on_update"] = (si.get("on_update") or []) + (
                        drop_sync.get("on_update") or [])
                    drop_sync = None
                if ins.get("opcode") == "Ldweights":
                    key = json.dumps(
                        [ins.get("ins"), ins.get("perf_mode"),
                         ins.get("is_transpose"), ins.get("tile_position")],
                        sort_keys=True)
                    if last_load == key:
                        si = ins.get("sync_info")
                        if si and (si.get("on_wait") or si.get("on_update")):
                            drop_sync = si
                        n_drop += 1
                        continue
                    last_load = key
                elif ins.get("engine") == "PE" and ins.get("opcode") != "Matmult":
                    last_load = None
                out.append(ins)
            assert drop_sync is None, "dropped Ldweights sync had no successor"
            blk["instructions"] = out
    return json.dumps(bir).encode()


def _install_compile_patch():
    """Route every BIR compile through the dedup + sync-split passes."""
    if _CACHE.get("patched"):
        return
    import concourse.bass_utils as bu

    orig = bu.compile_bir_kernel

    def patched(bir_json, tmpdir, neff_name="file.neff"):
        if isinstance(bir_json, str):
            bir_json = bir_json.encode()
        return orig(_split_sync_waits(_dedup_ldweights(bir_json)),
                    tmpdir, neff_name)

    bu.compile_bir_kernel = patched
    try:
        import concourse.bass2jax as b2j

        b2j.compile_bir_kernel = patched
    except ImportError:
        pass
    _CACHE["patched"] = True


def _build():
    import concourse.bass as bass
    import concourse.mybir as mybir
    from concourse.tile import TileContext

    f32 = mybir.dt.float32
    bf16 = mybir.dt.bfloat16
    f16 = mybir.dt.float16
    nc = bass.Bass()

    x0t = nc.dram_tensor("x0t", [P, BH], bf16, kind="ExternalInput")
    x1t = nc.dram_tensor("x1t", [P, BH], bf16, kind="ExternalInput")
    x1n = nc.dram_tensor("x1n", [BH, P], f32, kind="ExternalInput")
    tcm = nc.dram_tensor("tcm", [P, 128 * 128], bf16, kind="ExternalInput")
    tuv = nc.dram_tensor("tuv", [P, 256], bf16, kind="ExternalInput")
    tcc = nc.dram_tensor("tcc", [1, P], bf16, kind="ExternalInput")
    out = nc.dram_tensor("out", [BH, P], f32, kind="ExternalOutput")

    mult = mybir.AluOpType.mult
    add = mybir.AluOpType.add

    def grp3(ap, n_grp, inner):
        # [P, n_grp*inner] contiguous 2D AP -> 3D [P, n_grp, inner]
        return bass.AP(ap.tensor, ap.offset,
                       [list(ap.ap[0]), [inner, n_grp], [1, inner]])

    def rep_inner(ap, n_grp, n_rep):
        # [P, n_grp] 2D AP -> 3D [P, n_grp, n_rep] with innermost stride 0
        return bass.AP(ap.tensor, ap.offset,
                       [list(ap.ap[0]), [1, n_grp], [0, n_rep]])

    idn = nc.dram_tensor("idn", [P, P], bf16, kind="ExternalInput")

    with TileContext(nc) as tc:
        with (
            tc.tile_pool(name="const", bufs=1) as cpool,
            tc.tile_pool(name="rpsum", bufs=2, space="PSUM") as ppool,
            tc.tile_pool(name="apsum", bufs=2, space="PSUM") as accpool,
            tc.tile_pool(name="bpsum", bufs=1, space="PSUM") as acc2pool,
            tc.tile_pool(name="bcx", bufs=3) as bcxpool,
            tc.tile_pool(name="xb", bufs=2) as xpool,
            tc.tile_pool(name="zbuf", bufs=2) as zpool,
            tc.tile_pool(name="bev", bufs=2) as bevpool,
            tc.tile_pool(name="outb", bufs=2) as opool,
            tc.tile_pool(name="outt", bufs=16) as otpool,
        ):
            # --- resident constants (order = first-consumption order) ---
            # x0t/x1t split into 4 DMAs so the first beta mult isn't gated
            # on one 512 KiB serial transfer
            x0t_s = cpool.tile([P, BH], bf16, tag="x0t")
            for q in range(4):
                nc.sync.dma_start(x0t_s[:, q * 512 : (q + 1) * 512],
                                  x0t[:, q * 512 : (q + 1) * 512])
            x1t_s = cpool.tile([P, BH], bf16, tag="x1t")
            for q in range(4):
                nc.sync.dma_start(x1t_s[:, q * 512 : (q + 1) * 512],
                                  x1t[:, q * 512 : (q + 1) * 512])
            # first two broadcast groups issue before the resident bulk
            # so the beta stream starts immediately
            bcx_tiles = {}

            def issue_bcx(jp):
                # broadcast rows 2jp, 2jp+1 (8 KiB contiguous in HBM) to
                # all 128 partitions: one descriptor per partition
                j0 = 2 * jp
                bcx = bcxpool.tile([P, 2 * BH], bf16, tag="bcx",
                                   name=f"bcx_{jp}")
                srcr = x1t[j0 : j0 + 2, :]
                nc.sync.dma_start(
                    bcx, bass.AP(srcr.tensor, srcr.offset,
                                 [[0, P], [1, 2 * BH]]))
                bcx_tiles[jp] = bcx

            issue_bcx(0)
            issue_bcx(1)
            issue_bcx(2)
            tuv_s = cpool.tile([P, 256], bf16, tag="tuv")
            nc.sync.dma_start(tuv_s, tuv[:, :])
            tcc_s = cpool.tile([1, P], bf16, tag="tcc")
            nc.sync.dma_start(tcc_s, tcc[:, :])
            idn_s = cpool.tile([P, P], bf16, tag="idn")
            nc.sync.dma_start(idn_s, idn[:, :])
            ones_s = cpool.tile([1, P], bf16, tag="ones")
            nc.vector.memset(ones_s, 1.0)
            tcm_tiles = []
            for g in range(16):
                tg = cpool.tile([P, 1024], bf16, tag=f"tcm{g}",
                                name=f"tcm_t{g}")
                tcm_tiles.append(tg)
            # alpha's T tiles (12-15) + beta's first tiles load first so
            # neither stream stalls at kernel start
            for g in (12, 13, 14, 15, 0, 1, 2):
                nc.sync.dma_start(tcm_tiles[g],
                                  tcm[:, g * 1024 : (g + 1) * 1024])
            x1n_all = cpool.tile([P, NT * P], f32, tag="x1n_all")
            x1n_full = x1n[:, :]
            for q in range(4):
                nc.sync.dma_start(
                    grp3(x1n_all[:, q * 4 * P : (q + 1) * 4 * P], 4, P),
                    bass.AP(x1n_full.tensor,
                            x1n_full.offset + q * 4 * P * P,
                            [[P, P], [P * P, 4], [1, P]]))
            x1b_all = cpool.tile([P, NT * P], bf16, tag="x1b_all")
            nc.vector.tensor_copy(x1b_all, x1n_all)
            for g in range(3, 12):
                nc.sync.dma_start(tcm_tiles[g],
                                  tcm[:, g * 1024 : (g + 1) * 1024])

            pending = []
            out_ts = {}

            def pairs(ap_base):
                # [P, >=384] AP -> 3D [P, 2, 128] picking 128-col blocks at
                # +0 and +256 (interleaved quarters of the id-matmul acc)
                return bass.AP(ap_base.tensor, ap_base.offset,
                               [list(ap_base.ap[0]), [256, 2], [1, P]])

            # ================= beta path (j in [0, NJB)) =================
            # psum_b[o, b-block] += sum_i T[i,j,o] * (x0T (.) bcast(x1T[j]))
            # j-outer: one full-width broadcast DMA (4 KiB/partition lines),
            # one wide 2x-mode VectorE mult, 4 matmuls per stationary load.
            # The j-sum accumulates in 4 persistent PSUM banks; DVE/ScalarE
            # never touch the 34M-element intermediate.
            acc2 = acc2pool.tile([P, NB * 512], f32, tag="acc2")

            def beta_step(jp):
                if jp + 3 < NJB // 2:
                    issue_bcx(jp + 3)
                bcx = bcx_tiles.pop(jp)
                x4 = xpool.tile([P, 2 * BH], bf16, tag="x4", name=f"x4_{jp}")
                x0full = x0t_s[:, :]
                x0rep = bass.AP(x0full.tensor, x0full.offset,
                                [list(x0full.ap[0]), [0, 2], [1, BH]])
                nc.vector.tensor_tensor(grp3(x4[:, :], 2, BH), x0rep,
                                        grp3(bcx[:, :], 2, BH), mult)
                for jj in range(2):
                    j = 2 * jp + jj
                    for B in range(NB):
                        nc.tensor.matmul(
                            acc2[:, B * 512 : (B + 1) * 512],
                            tcm_tiles[j // 8][
                                :, (j % 8) * 128 : (j % 8) * 128 + 128],
                            x4[:, jj * BH + B * 512 : jj * BH + (B + 1) * 512],
                            start=(j == 0), stop=(j == NJB - 1),
                        )

            # ================= alpha path (j in [NJB, 128)) ==============
            def alpha_phases(t):
                bsl = slice(t * P, (t + 1) * P)
                x1n_t = x1n_all[:, bsl]
                x1b_t = x1b_all[:, bsl]
                zb = zpool.tile([P, NJA * P], bf16, tag="zbuf",
                                name=f"zbuf_{t}")
                acc = accpool.tile([P, 512], f32, tag="acc", name=f"acc_{t}")
                tmp = opool.tile([P, 2 * P], f32, tag="ctmp",
                                 name=f"ctmp_{t}")
                tmp2 = opool.tile([P, 2 * P], f32, tag="ctmp2",
                                  name=f"ctmp2_{t}")
                out_t = otpool.tile([P, P], f32, tag="out_t",
                                    name=f"out_{t}")
                out_ts[t] = out_t
                phases = []

                def ph_uv():
                    nc.tensor.matmul(acc[:, 0:P], ones_s, tcc_s,
                                     start=True, stop=False)
                    nc.tensor.matmul(acc[:, 0:P], x1t_s[:, bsl],
                                     tuv_s[:, 128:256], start=False,
                                     stop=False)
                    nc.tensor.matmul(acc[:, 0:P], x0t_s[:, bsl],
                                     tuv_s[:, 0:128], start=False, stop=False)
                phases.append(ph_uv)

                def ph_unit(u):
                    # 4-j unit: one 512-wide stage-1 matmul + per-j scale
                    def f():
                        r = ppool.tile([P, 512], f32, tag="r",
                                       name=f"r_{t}_{u}")
                        col0 = NJB * P + u * 512
                        nc.tensor.matmul(
                            r, x0t_s[:, bsl],
                            tcm_tiles[col0 // 1024][
                                :, col0 % 1024 : col0 % 1024 + 512],
                            start=True, stop=True,
                        )
                        j0 = 4 * u
                        if u in ACT_UNITS_A:
                            for jj in range(4):
                                nc.scalar.activation(
                                    zb[:, (j0 + jj) * P : (j0 + jj + 1) * P],
                                    r[:, jj * P : (jj + 1) * P],
                                    mybir.ActivationFunctionType.Copy,
                                    scale=x1n_t[:, NJB + j0 + jj
                                                : NJB + j0 + jj + 1],
                                )
                        else:
                            nc.vector.tensor_tensor(
                                grp3(zb[:, j0 * P : (j0 + 4) * P], 4, P),
                                grp3(r[:, :], 4, P),
                                rep_inner(x1b_t[:, NJB + j0 : NJB + j0 + 4],
                                          4, P),
                                mult,
                            )
                    return f
                for u in range(NUA):
                    phases.append(ph_unit(u))

                def ph_ids():
                    for g in range(NJA // 4):
                        nc.tensor.matmul(
                            acc[:, 0:512], idn_s,
                            zb[:, g * 512 : (g + 1) * 512],
                            start=False, stop=(g == NJA // 4 - 1),
                        )
                phases.append(ph_ids)

                def ph_c13():
                    nc.scalar.activation(
                        grp3(tmp[:, :], 2, P), pairs(acc[:, P : 4 * P]),
                        mybir.ActivationFunctionType.Copy)

                def ph_c02():
                    nc.vector.tensor_tensor(
                        grp3(tmp2[:, :], 2, P), pairs(acc[:, 0 : 3 * P]),
                        grp3(tmp[:, :], 2, P), add)

                def ph_cfa():
                    nc.vector.tensor_add(out_t, tmp2[:, 0:P],
                                         tmp2[:, P : 2 * P])
                phases.append(ph_c13)
                phases.append(ph_c02)
                phases.append(ph_cfa)
                return phases

            # interleave the beta j-stream with the alpha tile phases
            aops = []
            for t in range(NT):
                aops += alpha_phases(t)
            # interleave alpha phases into the beta j-pair stream, leaving
            # ~16 alpha phases to trail so they overlap the beta-evac chain
            na, nb_ = len(aops), NJB // 2
            ai = 0
            for jp in range(nb_):
                beta_step(jp)
                target = (jp + 1) * max(na - 16, 1) // nb_
                while ai < target:
                    aops[ai]()
                    ai += 1

            # beta evacuation first (ScalarE copy + xbar transpose run while
            # the trailing alpha phases execute), adds + output DMAs last
            tbs = []
            for B in range(NB):
                ob = bevpool.tile([P, 512], bf16, tag="bev", name=f"bev_{B}")
                tb = bevpool.tile([P, 512], bf16, tag="tbt", name=f"tbt_{B}")
                nc.scalar.activation(ob, acc2[:, B * 512 : (B + 1) * 512],
                                     mybir.ActivationFunctionType.Copy)
                for tt in range(4):
                    nc.sync.dma_start(tb[:, tt * P : (tt + 1) * P],
                                      ob[:, tt * P : (tt + 1) * P],
                                      transpose=True)
                tbs.append(tb)
            while ai < na:
                aops[ai]()
                ai += 1
            for B in range(NB):
                for tt in range(4):
                    t = 4 * B + tt
                    nc.vector.tensor_add(
                        out_ts[t], out_ts[t],
                        tbs[B][:, tt * P : (tt + 1) * P])
                    nc.sync.dma_start(out[t * P : (t + 1) * P, :], out_ts[t])

    return nc



def _get_nc():
    if "nc" not in _CACHE:
        _CACHE["nc"] = _build()
    return _CACHE["nc"]


def _make_runner(nc):
    """Persistent sharded-jit runner for the axon/PJRT path (specialized copy
    of bass2jax.run_bass_via_pjrt so repeated calls reuse one compiled
    executable). Returns run(in_maps) -> list[dict[str, np.ndarray]]."""
    import jax
    import numpy as jnp_np  # noqa
    from jax.sharding import Mesh, PartitionSpec
    from jax.experimental.shard_map import shard_map
    import concourse.mybir as mybir
    from concourse.bass2jax import (
        _bass_exec_p, install_neuronx_cc_hook, partition_id_tensor)

    install_neuronx_cc_hook()

    partition_name = nc.partition_id_tensor.name if nc.partition_id_tensor else None
    in_names, out_names, out_avals, zero_outs = [], [], [], []
    for alloc in nc.m.functions[0].allocations:
        if not isinstance(alloc, mybir.MemoryLocationSet):
            continue
        name = alloc.memorylocations[0].name
        if alloc.kind == "ExternalInput":
            if name != partition_name:
                in_names.append(name)
        elif alloc.kind == "ExternalOutput":
            out_names.append(name)
            shape = tuple(alloc.tensor_shape)
            dtype = mybir.dt.np(alloc.dtype)
            out_avals.append(jax.core.ShapedArray(shape, dtype))
            zero_outs.append(np.zeros(shape, dtype))
    n_params = len(in_names)
    n_outs = len(out_avals)
    all_in_names = list(in_names) + list(out_names)
    if partition_name is not None:
        all_in_names.append(partition_name)
    donate = tuple(range(n_params, n_params + n_outs))

    def _body(*args):
        operands = list(args)
        if partition_name is not None:
            operands.append(partition_id_tensor())
        outs = _bass_exec_p.bind(
            *operands,
            out_avals=tuple(out_avals),
            in_names=tuple(all_in_names),
            out_names=tuple(out_names),
            lowering_input_output_aliases=(),
            sim_require_finite=True,
            sim_require_nnan=True,
            nc=nc,
        )
        return tuple(outs)

    devices = jax.devices()[:NCORES]
    mesh = Mesh(np.asarray(devices), ("core",))
    in_specs = (PartitionSpec("core"),) * (n_params + n_outs)
    out_specs = (PartitionSpec("core"),) * len(out_names)
    sharded = jax.jit(
        shard_map(_body, mesh=mesh, in_specs=in_specs, out_specs=out_specs,
                  check_rep=False),
        donate_argnums=donate, keep_unused=True)

    def run(in_maps, raw=False):
        concat_in = [
            np.concatenate([np.asarray(m[name]) for m in in_maps], axis=0)
            for name in in_names
        ]
        concat_zeros = [
            np.zeros((NCORES * z.shape[0], *z.shape[1:]), z.dtype)
            for z in zero_outs
        ]
        out_arrs = sharded(*concat_in, *concat_zeros)
        if raw:
            return out_arrs
        return [
            {name: np.asarray(out_arrs[i]).reshape(NCORES, *out_avals[i].shape)[c]
             for i, name in enumerate(out_names)}
            for c in range(NCORES)
        ]

    return run


def _run(nc, in_maps):
    """Execute on 8 cores; under axon go through the persistent PJRT runner."""
    from concourse._compat import axon_active

    _install_compile_patch()

    if axon_active():
        if "runner" not in _CACHE:
            _CACHE["runner"] = _make_runner(nc)
        return _CACHE["runner"](in_maps), None

    from concourse.bass_utils import run_bass_kernel_spmd

    res = run_bass_kernel_spmd(nc, in_maps, core_ids=list(range(NCORES)))
    return res.results, res


def _make_in_maps(x0, x1, T):
    import ml_dtypes

    bf16 = ml_dtypes.bfloat16
    x0 = np.asarray(x0, dtype=np.float32)
    x1 = np.asarray(x1, dtype=np.float32)
    T = np.asarray(T, dtype=np.float32)

    idn = np.eye(P, dtype=np.float32).astype(bf16)
    in_maps = []
    for c in range(NCORES):
        a, h = divmod(c, 2)
        bsl = slice(h * BH, (h + 1) * BH)
        x0c = np.ascontiguousarray(x0[bsl, a, :])  # (BH, 128)
        x1c = np.ascontiguousarray(x1[bsl, a, :])
        in_maps.append({
            "x0t": np.ascontiguousarray(x0c.T).astype(bf16),
            "x1t": np.ascontiguousarray(x1c.T).astype(bf16),
            "x1n": x1c,
            "tcm": np.ascontiguousarray(
                T[a, :128, :128, :].reshape(128, 128 * 128)).astype(bf16),
            "tuv": np.ascontiguousarray(
                np.concatenate([T[a, :128, 128, :], T[a, 128, :128, :]],
                               axis=1)).astype(bf16),
            "tcc": np.ascontiguousarray(
                T[a, 128, 128, :].reshape(1, 128)).astype(bf16),
            "idn": idn,
        })
    return in_maps


def kernel(x0, x1, T):
    global LAST_RESULT

    in_maps = _make_in_maps(x0, x1, T)
    nc = _get_nc()
    results, LAST_RESULT = _run(nc, in_maps)

    full = np.empty((BS, A, OUT), dtype=np.float32)
    for c in range(NCORES):
        a, h = divmod(c, 2)
        full[h * BH : (h + 1) * BH, a, :] = results[c]["out"]
    return full

